# revision 22
# baseline (speedup 1.0000x reference)
"""Trainium2 Bass kernel: causal multi-head self-attention.

Problem: B=2, T=4096, C=768, H=12, D=64, causal softmax(QK^T/sqrt(D))V + out proj.

Sharding (8 cores): core c handles batch b=c//4 and 3 heads g=c%4 (rows
192*g:192*(g+1) of wq/wk/wv, same columns of wo). Each core computes its
heads' full attention and a partial out-projection (T, C) for its batch;
the host sums the 4 partials per batch (output is sum-sharded across the
tensor-parallel group) and transposes back to (B, T, C).

Device-side dataflow (everything stays transposed; zero on-chip transposes):
  - inputs are fed pre-transposed: xT (C, T), wqT/wkT/wvT (C, 192), woT (256, C)
  - Q^T, K^T (d on partitions) and V (t on partitions) via matmuls over xT
  - scores computed transposed: S^T[l, q] = sum_d K^T[d,l] Q^T[d,q] (PSUM)
  - every matmul keeps contraction K=128 (zero-padded Q / woT rows): the PE
    HAM clock monitor ignores partial-contraction matmuls, so K=64 work runs
    at 1.2 GHz instead of 2.4 GHz — padding with zeros is 2x faster
  - causal: only l-tiles up to the diagonal; diagonal exp tiles multiplied
    by a 0/1 mask on GpSimd (idle engine) after exp
  - exp on ScalarE with scale=1/sqrt(D) folded in; no max-subtraction
    (scores ~ N(0,1), fp32 exp is exact-safe and matches softmax math)
  - ctx^T[dv, q] = sum_l V[l, dv] E[l, q] accumulated in PSUM; V carries an
    extra ones column so row 64 of the PSUM accumulator is the softmax
    denominator; normalize with reciprocal_approx_fast
  - out^T[oc, t] = sum_dh woT[dh, oc] ctx^T[dh, t]

All matmul operands are float32r (full-rate fp32, ~tf32 operand rounding).
"""

import os
import sys
import types

import numpy as np

if "/opt/trn_rl_repo" not in sys.path:
    sys.path.insert(0, "/opt/trn_rl_repo")

import concourse.bass as bass  # noqa: E402
import concourse.mybir as mybir  # noqa: E402
from concourse import bacc, tile  # noqa: E402
from concourse.bass_utils import run_bass_kernel_spmd  # noqa: E402

F32 = mybir.dt.float32
F32R = mybir.dt.float32r
BF16 = mybir.dt.bfloat16
EXP = mybir.ActivationFunctionType.Exp

B, T, C, H, D = 2, 4096, 768, 12, 64
HPD = 3          # heads per device
DH = HPD * D     # 192 local head channels
NCORES = 8
QB = 512         # query block (matmul free dim / PSUM bank)
LT = 128         # key(l)-tile size
GRP = 3          # l-tiles per exp group (3 PSUM banks)


def build_kernel(t=T, trace_sim=False):
    n_lt = t // LT
    n_qb = t // QB
    n_ch = t // QB
    nct = C // 128            # 6

    nc = bacc.Bacc("TRN2", target_bir_lowering=False, debug=False,
                   num_devices=NCORES)
    xT_d = nc.dram_tensor("xT", [C, t], F32R, kind="ExternalInput")
    wqT_d = nc.dram_tensor("wqT", [C, DH], F32R, kind="ExternalInput")
    wkT_d = nc.dram_tensor("wkT", [C, DH], F32R, kind="ExternalInput")
    wvT_d = nc.dram_tensor("wvT", [C, 256], F32R, kind="ExternalInput")  # padded
    woT_d = nc.dram_tensor("woT", [256, C], F32R, kind="ExternalInput")  # padded
    outT_d = nc.dram_tensor("outT", [C, t], F32, kind="ExternalOutput")

    with tile.TileContext(nc, trace_sim=trace_sim) as tc:
        with (
            tc.tile_pool(name="const", bufs=1) as const,
            tc.tile_pool(name="xs", bufs=2) as xs,
            tc.tile_pool(name="epool", bufs=2) as epool,
            tc.tile_pool(name="small", bufs=2) as small,
            tc.tile_pool(name="sp", bufs=2, space="PSUM") as sp,
            tc.tile_pool(name="ps", bufs=2, space="PSUM") as ps,
        ):
            # ---- weights / constants -------------------------------------
            wqT_s = const.tile([128, nct, DH], F32R)
            wkT_s = const.tile([128, nct, DH], F32R)
            wvT_s = const.tile([128, nct, 256], F32R)
            nc.sync.dma_start(wqT_s[:], wqT_d.ap().rearrange("(ct p) d -> p ct d", p=128))
            nc.sync.dma_start(wkT_s[:], wkT_d.ap().rearrange("(ct p) d -> p ct d", p=128))
            nc.sync.dma_start(wvT_s[:], wvT_d.ap().rearrange("(ct p) d -> p ct d", p=128))
            woT_a = const.tile([128, C], F32R)
            woT_b = const.tile([128, C], F32R)   # rows 64:128 are host zeros
            nc.sync.dma_start(woT_a[:], woT_d.ap()[0:128, :])
            nc.sync.dma_start(woT_b[:], woT_d.ap()[128:256, :])

            # additive diagonal causal masks: mask[k][p, f] = 0 if p + 128k <= f
            masks = []
            for k in range(QB // LT):
                m = const.tile([128, QB], F32, tag=f"mask{k}")
                nc.gpsimd.memset(m[:], 0.0)
                nc.gpsimd.affine_select(
                    out=m[:], in_=m[:],
                    compare_op=mybir.AluOpType.is_ge,
                    fill=-1.0e30,
                    base=-128 * k,
                    channel_multiplier=-1,
                    pattern=[[1, QB]],
                )
                mb = const.tile([128, QB], BF16, tag=f"maskb{k}", name=f"maskb{k}")
                nc.vector.tensor_copy(mb[:], m[:])
                masks.append(mb)

            ident = const.tile([128, 128], F32, tag="identf")
            nc.gpsimd.memset(ident[:], 1.0)
            nc.gpsimd.affine_select(
                out=ident[:], in_=ident[:],
                compare_op=mybir.AluOpType.is_equal,
                fill=0.0, base=0, channel_multiplier=1,
                pattern=[[-1, 128]],
            )
            identb = const.tile([128, 128], BF16, tag="identb")
            nc.vector.tensor_copy(identb[:], ident[:])

            ones1 = const.tile([128, 1], F32)
            nc.vector.memset(ones1[:], 1.0)
            zero1 = const.tile([128, 1], F32)
            nc.vector.memset(zero1[:], 0.0)

            # ---- big persistent activations ------------------------------
            # K: heads 0,1 stacked; head 2 alone (top half zeroed)
            KT01 = const.tile([128, t], BF16)
            KT2 = const.tile([128, t], BF16)
            # Q: one tile per head, other-head partition rows zeroed so every
            # scores matmul contracts over the full 128 partitions
            QTz = [const.tile([128, t], BF16, tag=f"qtz{h}", name=f"qtz{h}")
                   for h in range(HPD)]
            # V with ones column per head: [128, n_lt, 3*65]
            Vone = const.tile([128, n_lt, HPD * 65], BF16)
            ctxT01 = const.tile([128, t], F32R)
            ctxT2 = const.tile([128, t], F32R)   # rows 64:128 zeroed

            # zero-fill everything with dead rows (avoids NaN*0 in the PE)
            for buf in (*QTz, KT2, ctxT2):
                nc.vector.tensor_copy(buf[:], zero1[:].to_broadcast((128, t)))
            nc.vector.tensor_copy(
                Vone[:].rearrange("p a b -> p (a b)"),
                ones1[:].to_broadcast((128, n_lt * HPD * 65)))

            # live partition rows per head for Q/ctx
            qrows = [slice(0, 64), slice(64, 128), slice(0, 64)]

            # ---- phase 1: projections ------------------------------------
            xT_r = xT_d.ap().rearrange("(ct p) t -> p ct t", p=128)
            for ch in range(n_ch):
                cs = slice(ch * QB, (ch + 1) * QB)
                xc = xs.tile([128, nct, QB], F32R)
                for ct in range(nct):
                    nc.sync.dma_start(xc[:, ct, :], xT_r[:, ct, cs])

                pqk = sp.tile([128, 3 * QB], F32, tag="sp")
                for ct in range(nct):
                    f, l = (ct == 0), (ct == nct - 1)
                    nc.tensor.matmul(pqk[:, 0:QB], wqT_s[:, ct, 0:128],
                                     xc[:, ct, :], start=f, stop=l)
                    nc.tensor.matmul(pqk[:, QB:2 * QB], wkT_s[:, ct, 0:128],
                                     xc[:, ct, :], start=f, stop=l)
                    nc.tensor.matmul(pqk[0:64, 2 * QB:3 * QB], wqT_s[:, ct, 128:DH],
                                     xc[:, ct, :], start=f, stop=l)
                pk2 = ps.tile([64, QB], F32, tag="ps")
                for ct in range(nct):
                    nc.tensor.matmul(pk2[:], wkT_s[:, ct, 128:DH],
                                     xc[:, ct, :], start=(ct == 0),
                                     stop=(ct == nct - 1))

                # copy projections out of PSUM (all lane-aligned)
                nc.vector.tensor_copy(QTz[0][0:64, cs], pqk[0:64, 0:QB])
                nc.vector.tensor_copy(QTz[1][64:128, cs], pqk[64:128, 0:QB])
                nc.vector.tensor_copy(QTz[2][0:64, cs], pqk[0:64, 2 * QB:3 * QB])
                nc.vector.tensor_copy(KT01[:, cs], pqk[:, QB:2 * QB])
                nc.vector.tensor_copy(KT2[0:64, cs], pk2[:])

                # V natural layout: stationary xT block, stream wvT (padded 256)
                pv = sp.tile([128, 3 * QB], F32, tag="sp")
                for ts in range(QB // 128):
                    tt = ch * (QB // 128) + ts
                    po = pv[:, ts * 256:(ts + 1) * 256]
                    for ct in range(nct):
                        nc.tensor.matmul(po, xc[:, ct, ts * 128:(ts + 1) * 128],
                                         wvT_s[:, ct, :], start=(ct == 0),
                                         stop=(ct == nct - 1))
                    for h in range(HPD):
                        nc.vector.tensor_copy(
                            Vone[:, tt, h * 65:h * 65 + 64],
                            pv[:, ts * 256 + h * 64:ts * 256 + (h + 1) * 64])

            # ---- phase 2: attention + interleaved output projection -------
            def emit_outproj(qb, oc):
                qs = slice(qb * QB, (qb + 1) * QB)
                ocs = slice(oc * 128, (oc + 1) * 128)
                po = ps.tile([128, QB], F32, tag="ps")
                nc.tensor.matmul(po[:], woT_a[:, ocs], ctxT01[:, qs],
                                 start=True, stop=False)
                nc.tensor.matmul(po[:], woT_b[:, ocs], ctxT2[:, qs],
                                 start=False, stop=True)
                ot = small.tile([128, QB], F32, tag="ot")
                nc.vector.tensor_copy(ot[:], po[:])
                nc.sync.dma_start(outT_d.ap()[ocs, qs], ot[:])

            pending = []
            for qb in range(n_qb):
                qs = slice(qb * QB, (qb + 1) * QB)
                L = (qb + 1) * (QB // LT)
                for h in range(HPD):
                    KT_h = KT01 if h < 2 else KT2
                    ctxp = ps.tile([65, QB], F32, tag="ps")
                    for g0 in range(0, L, GRP):
                        gl = min(GRP, L - g0)
                        spt = sp.tile([128, 3 * QB], F32, tag="sp")
                        for i in range(gl):
                            lt = g0 + i
                            sl = spt[:, i * QB:(i + 1) * QB]
                            diag = lt - qb * (QB // LT)
                            if diag >= 0:
                                # causal mask as a PE pre-accumulation:
                                # identity.T @ maskb = the -1e30 step pattern
                                nc.tensor.matmul(sl, identb[:], masks[diag][:],
                                                 start=True, stop=False)
                                nc.tensor.matmul(sl,
                                                 KT_h[:, lt * LT:(lt + 1) * LT],
                                                 QTz[h][:, qs],
                                                 start=False, stop=True)
                            else:
                                nc.tensor.matmul(sl,
                                                 KT_h[:, lt * LT:(lt + 1) * LT],
                                                 QTz[h][:, qs],
                                                 start=True, stop=True)
                        et = epool.tile([128, GRP * QB], BF16)
                        nc.scalar.activation(et[:, :gl * QB], spt[:, :gl * QB],
                                             EXP, scale=0.125)
                        for i in range(gl):
                            lt = g0 + i
                            nc.tensor.matmul(ctxp[:],
                                             Vone[:, lt, h * 65:h * 65 + 65],
                                             et[:, i * QB:(i + 1) * QB],
                                             start=(lt == 0), stop=(lt == L - 1))
                        if pending:
                            emit_outproj(*pending.pop(0))
                    # free the PSUM accumulator immediately, normalize off-path
                    stg = small.tile([128, QB], F32, tag="stg")
                    nc.vector.tensor_copy(stg[0:65, :], ctxp[0:65, :])
                    dn = small.tile([1, QB], F32, tag="dn")
                    nc.vector.tensor_copy(dn[:], stg[64:65, :])
                    rec = small.tile([1, QB], F32, tag="rec")
                    nc.vector.reciprocal_approx_fast(rec[:], dn[:])
                    rb = small.tile([64, QB], F32, tag="rb")
                    nc.gpsimd.partition_broadcast(rb[:], rec[:])
                    if h == 1:
                        st2 = small.tile([64, QB], F32R, tag="st2")
                        nc.gpsimd.tensor_tensor(st2[:], stg[0:64, :], rb[:],
                                                mybir.AluOpType.mult)
                        nc.sync.dma_start(ctxT01[64:128, qs], st2[:])
                    else:
                        dst = ctxT01 if h == 0 else ctxT2
                        nc.gpsimd.tensor_tensor(dst[0:64, qs], stg[0:64, :],
                                                rb[:], mybir.AluOpType.mult)
                pending.extend((qb, oc) for oc in range(nct))
            for item in pending:
                emit_outproj(*item)

    nc.compile()
    return nc


_NC_CACHE = {}
LAST_EXEC_NS = None
LAST_RES = None


def _get_nc():
    if "full" not in _NC_CACHE:
        _NC_CACHE["full"] = build_kernel()
    return _NC_CACHE["full"]


def _install_ntff_shim():
    """Make run_bass_kernel_spmd(trace=True) work under axon in this image."""
    import antenv
    if "antenv.axon_hooks" in sys.modules:
        return
    mod = types.ModuleType("antenv.axon_hooks")
    mod._hook = None
    mod.set_axon_ntff_profile_hook = lambda h: setattr(mod, "_hook", h)
    mod.get_axon_ntff_profile_hook = lambda: mod._hook
    sys.modules["antenv.axon_hooks"] = mod
    antenv.axon_hooks = mod
    try:
        from trn_agent_boot.trn_boot import _ntff_profile_via_ctypes
        mod.set_axon_ntff_profile_hook(
            _ntff_profile_via_ctypes("/opt/axon/libaxon_pjrt.so"))
    except Exception:
        pass


def make_in_maps(x, wq, wk, wv, wo):
    x = np.asarray(x, dtype=np.float32)
    wq = np.asarray(wq, dtype=np.float32)
    wk = np.asarray(wk, dtype=np.float32)
    wv = np.asarray(wv, dtype=np.float32)
    wo = np.asarray(wo, dtype=np.float32)
    in_maps = []
    for c in range(NCORES):
        b, g = c // (NCORES // B), c % (NCORES // B)
        rs, re = g * DH, (g + 1) * DH
        wvT = np.zeros((C, 256), dtype=np.float32)
        wvT[:, :DH] = wv[rs:re].T
        woT = np.zeros((256, C), dtype=np.float32)
        woT[:DH] = wo[:, rs:re].T
        in_maps.append({
            "xT": np.ascontiguousarray(x[b].T),
            "wqT": np.ascontiguousarray(wq[rs:re].T),
            "wkT": np.ascontiguousarray(wk[rs:re].T),
            "wvT": wvT,
            "woT": woT,
        })
    return in_maps


def kernel(x, wq, wk, wv, wo):
    global LAST_EXEC_NS, LAST_RES
    in_maps = make_in_maps(x, wq, wk, wv, wo)
    nc = _get_nc()
    trace = bool(int(os.environ.get("KERNEL_TRACE", "0")))
    if trace:
        _install_ntff_shim()
    res = run_bass_kernel_spmd(nc, in_maps, core_ids=list(range(NCORES)),
                               trace=trace)
    LAST_EXEC_NS = res.exec_time_ns
    LAST_RES = res
    outT = [res.results[c]["outT"] for c in range(NCORES)]
    halves = []
    for b in range(B):
        acc = outT[4 * b].astype(np.float64)
        for c in range(4 * b + 1, 4 * b + 4):
            acc = acc + outT[c]
        halves.append(acc.T)
    return np.stack(halves).astype(np.float32)


# revision 23
# speedup vs baseline: 1.1856x; 1.1856x over previous
"""Trainium2 Bass kernel: causal multi-head self-attention.

Problem: B=2, T=4096, C=768, H=12, D=64, causal softmax(QK^T/sqrt(D))V + out proj.

Sharding (8 cores): core c handles batch b=c//4 and 3 heads g=c%4 (rows
192*g:192*(g+1) of wq/wk/wv, same columns of wo). Each core computes its
heads' full attention and a partial out-projection (T, C) for its batch;
the host sums the 4 partials per batch (output is sum-sharded across the
tensor-parallel group) and transposes back to (B, T, C).

Device-side dataflow (everything stays transposed; zero on-chip transposes):
  - inputs are fed pre-transposed: xT (C, T), wqT/wkT/wvT (C, 192), woT (256, C)
  - Q^T, K^T (d on partitions) and V (t on partitions) via matmuls over xT
  - scores computed transposed: S^T[l, q] = sum_d K^T[d,l] Q^T[d,q] (PSUM)
  - every matmul keeps contraction K=128 (zero-padded Q / woT rows): the PE
    HAM clock monitor ignores partial-contraction matmuls, so K=64 work runs
    at 1.2 GHz instead of 2.4 GHz — padding with zeros is 2x faster
  - causal: only l-tiles up to the diagonal; diagonal exp tiles multiplied
    by a 0/1 mask on GpSimd (idle engine) after exp
  - exp on ScalarE with scale=1/sqrt(D) folded in; no max-subtraction
    (scores ~ N(0,1), fp32 exp is exact-safe and matches softmax math)
  - ctx^T[dv, q] = sum_l V[l, dv] E[l, q] accumulated in PSUM; V carries an
    extra ones column so row 64 of the PSUM accumulator is the softmax
    denominator; normalize with reciprocal_approx_fast
  - out^T[oc, t] = sum_dh woT[dh, oc] ctx^T[dh, t]

All matmul operands are float32r (full-rate fp32, ~tf32 operand rounding).
"""

import os
import sys
import types

import numpy as np

if "/opt/trn_rl_repo" not in sys.path:
    sys.path.insert(0, "/opt/trn_rl_repo")

import concourse.bass as bass  # noqa: E402
import concourse.mybir as mybir  # noqa: E402
from concourse import bacc, tile  # noqa: E402
from concourse.bass_utils import run_bass_kernel_spmd  # noqa: E402

F32 = mybir.dt.float32
F32R = mybir.dt.float32r
BF16 = mybir.dt.bfloat16
EXP = mybir.ActivationFunctionType.Exp

B, T, C, H, D = 2, 4096, 768, 12, 64
HPD = 3          # heads per device
DH = HPD * D     # 192 local head channels
NCORES = 8
QB = 512         # query block (matmul free dim / PSUM bank)
LT = 128         # key(l)-tile size
GRP = 3          # l-tiles per exp group (3 PSUM banks)


def build_kernel(t=T, trace_sim=False):
    n_lt = t // LT
    n_qb = t // QB
    n_ch = t // QB
    nct = C // 128            # 6

    nc = bacc.Bacc("TRN2", target_bir_lowering=False, debug=False,
                   num_devices=NCORES)
    xT_d = nc.dram_tensor("xT", [C, t], F32R, kind="ExternalInput")
    wqT_d = nc.dram_tensor("wqT", [C, DH], F32R, kind="ExternalInput")
    wkT_d = nc.dram_tensor("wkT", [C, DH], F32R, kind="ExternalInput")
    wvT_d = nc.dram_tensor("wvT", [C, 256], F32R, kind="ExternalInput")  # padded
    woT_d = nc.dram_tensor("woT", [256, C], F32R, kind="ExternalInput")  # padded
    outT_d = nc.dram_tensor("outT", [C, t], F32, kind="ExternalOutput")

    with tile.TileContext(nc, trace_sim=trace_sim) as tc:
        with (
            tc.tile_pool(name="const", bufs=1) as const,
            tc.tile_pool(name="xs", bufs=2) as xs,
            tc.tile_pool(name="epool", bufs=2) as epool,
            tc.tile_pool(name="small", bufs=2) as small,
            tc.tile_pool(name="sp", bufs=2, space="PSUM") as sp,
            tc.tile_pool(name="ps", bufs=2, space="PSUM") as ps,
        ):
            # ---- weights / constants -------------------------------------
            wqT_s = const.tile([128, nct, DH], F32R)
            wkT_s = const.tile([128, nct, DH], F32R)
            wvT_s = const.tile([128, nct, 256], F32R)
            nc.sync.dma_start(wqT_s[:], wqT_d.ap().rearrange("(ct p) d -> p ct d", p=128))
            nc.sync.dma_start(wkT_s[:], wkT_d.ap().rearrange("(ct p) d -> p ct d", p=128))
            nc.sync.dma_start(wvT_s[:], wvT_d.ap().rearrange("(ct p) d -> p ct d", p=128))
            woT_a = const.tile([128, C], F32R)
            woT_b = const.tile([128, C], F32R)   # rows 64:128 are host zeros
            nc.sync.dma_start(woT_a[:], woT_d.ap()[0:128, :])
            nc.sync.dma_start(woT_b[:], woT_d.ap()[128:256, :])

            # additive diagonal causal masks: mask[k][p, f] = 0 if p + 128k <= f
            masks = []
            for k in range(QB // LT):
                m = const.tile([128, QB], F32, tag=f"mask{k}")
                nc.gpsimd.memset(m[:], 0.0)
                nc.gpsimd.affine_select(
                    out=m[:], in_=m[:],
                    compare_op=mybir.AluOpType.is_ge,
                    fill=-1.0e30,
                    base=-128 * k,
                    channel_multiplier=-1,
                    pattern=[[1, QB]],
                )
                mb = const.tile([128, QB], BF16, tag=f"maskb{k}", name=f"maskb{k}")
                nc.vector.tensor_copy(mb[:], m[:])
                masks.append(mb)

            ident = const.tile([128, 128], F32, tag="identf")
            nc.gpsimd.memset(ident[:], 1.0)
            nc.gpsimd.affine_select(
                out=ident[:], in_=ident[:],
                compare_op=mybir.AluOpType.is_equal,
                fill=0.0, base=0, channel_multiplier=1,
                pattern=[[-1, 128]],
            )
            identb = const.tile([128, 128], BF16, tag="identb")
            nc.vector.tensor_copy(identb[:], ident[:])

            ones1 = const.tile([128, 1], F32)
            nc.vector.memset(ones1[:], 1.0)
            zero1 = const.tile([128, 1], F32)
            nc.vector.memset(zero1[:], 0.0)

            # ---- big persistent activations ------------------------------
            # K: heads 0,1 stacked; head 2 alone (top half zeroed)
            KT01 = const.tile([128, t], BF16)
            KT2 = const.tile([128, t], BF16)
            # Q: one tile per head, other-head partition rows zeroed so every
            # scores matmul contracts over the full 128 partitions
            QTz = [const.tile([128, t], BF16, tag=f"qtz{h}", name=f"qtz{h}")
                   for h in range(HPD)]
            # V with ones column per head: [128, n_lt, 3*65]
            Vone = const.tile([128, n_lt, HPD * 65], BF16)
            ctxT01 = const.tile([128, t], F32R)
            ctxT2 = const.tile([128, t], F32R)   # rows 64:128 zeroed

            # zero-fill everything with dead rows (avoids NaN*0 in the PE)
            for buf in (*QTz, KT2, ctxT2):
                nc.vector.tensor_copy(buf[:], zero1[:].to_broadcast((128, t)))
            nc.vector.tensor_copy(
                Vone[:].rearrange("p a b -> p (a b)"),
                ones1[:].to_broadcast((128, n_lt * HPD * 65)))

            # live partition rows per head for Q/ctx
            qrows = [slice(0, 64), slice(64, 128), slice(0, 64)]

            # ---- phase 1: projections ------------------------------------
            xT_r = xT_d.ap().rearrange("(ct p) t -> p ct t", p=128)
            for ch in range(n_ch):
                cs = slice(ch * QB, (ch + 1) * QB)
                xc = xs.tile([128, nct, QB], F32R)
                for ct in range(nct):
                    nc.sync.dma_start(xc[:, ct, :], xT_r[:, ct, cs])

                pqk = sp.tile([128, 3 * QB], F32, tag="sp")
                for ct in range(nct):
                    f, l = (ct == 0), (ct == nct - 1)
                    nc.tensor.matmul(pqk[:, 0:QB], wqT_s[:, ct, 0:128],
                                     xc[:, ct, :], start=f, stop=l)
                    nc.tensor.matmul(pqk[:, QB:2 * QB], wkT_s[:, ct, 0:128],
                                     xc[:, ct, :], start=f, stop=l)
                    nc.tensor.matmul(pqk[0:64, 2 * QB:3 * QB], wqT_s[:, ct, 128:DH],
                                     xc[:, ct, :], start=f, stop=l)
                pk2 = ps.tile([64, QB], F32, tag="ps")
                for ct in range(nct):
                    nc.tensor.matmul(pk2[:], wkT_s[:, ct, 128:DH],
                                     xc[:, ct, :], start=(ct == 0),
                                     stop=(ct == nct - 1))

                # copy projections out of PSUM (all lane-aligned)
                nc.vector.tensor_copy(QTz[0][0:64, cs], pqk[0:64, 0:QB])
                nc.vector.tensor_copy(QTz[1][64:128, cs], pqk[64:128, 0:QB])
                nc.vector.tensor_copy(QTz[2][0:64, cs], pqk[0:64, 2 * QB:3 * QB])
                nc.vector.tensor_copy(KT01[:, cs], pqk[:, QB:2 * QB])
                nc.vector.tensor_copy(KT2[0:64, cs], pk2[:])

                # V natural layout: stationary xT block, stream wvT (padded 256)
                pv = sp.tile([128, 3 * QB], F32, tag="sp")
                for ts in range(QB // 128):
                    tt = ch * (QB // 128) + ts
                    po = pv[:, ts * 256:(ts + 1) * 256]
                    for ct in range(nct):
                        nc.tensor.matmul(po, xc[:, ct, ts * 128:(ts + 1) * 128],
                                         wvT_s[:, ct, :], start=(ct == 0),
                                         stop=(ct == nct - 1))
                    for h in range(HPD):
                        nc.vector.tensor_copy(
                            Vone[:, tt, h * 65:h * 65 + 64],
                            pv[:, ts * 256 + h * 64:ts * 256 + (h + 1) * 64])

            # ---- phase 2: attention + interleaved output projection -------
            def emit_outproj(qb, oc):
                qs = slice(qb * QB, (qb + 1) * QB)
                ocs = slice(oc * 128, (oc + 1) * 128)
                po = ps.tile([128, QB], F32, tag="ps")
                nc.tensor.matmul(po[:], woT_a[:, ocs], ctxT01[:, qs],
                                 start=True, stop=False)
                nc.tensor.matmul(po[:], woT_b[:, ocs], ctxT2[:, qs],
                                 start=False, stop=True)
                ot = small.tile([128, QB], F32, tag="ot")
                nc.vector.tensor_copy(ot[:], po[:])
                nc.sync.dma_start(outT_d.ap()[ocs, qs], ot[:])

            pending = []
            for qb in range(n_qb):
                qs = slice(qb * QB, (qb + 1) * QB)
                L = (qb + 1) * (QB // LT)
                for h in range(HPD):
                    KT_h = KT01 if h < 2 else KT2
                    ctxp = ps.tile([65, QB], F32, tag="ps")
                    for g0 in range(0, L, GRP):
                        gl = min(GRP, L - g0)
                        spt = sp.tile([128, 3 * QB], F32, tag="sp")
                        for i in range(gl):
                            lt = g0 + i
                            sl = spt[:, i * QB:(i + 1) * QB]
                            diag = lt - qb * (QB // LT)
                            if diag >= 0:
                                # causal mask as a PE pre-accumulation:
                                # identity.T @ maskb = the -1e30 step pattern
                                nc.tensor.matmul(sl, identb[:], masks[diag][:],
                                                 start=True, stop=False)
                                nc.tensor.matmul(sl,
                                                 KT_h[:, lt * LT:(lt + 1) * LT],
                                                 QTz[h][:, qs],
                                                 start=False, stop=True)
                            else:
                                nc.tensor.matmul(sl,
                                                 KT_h[:, lt * LT:(lt + 1) * LT],
                                                 QTz[h][:, qs],
                                                 start=True, stop=True)
                        et = epool.tile([128, GRP * QB], BF16)
                        nc.scalar.activation(et[:, :gl * QB], spt[:, :gl * QB],
                                             EXP, scale=0.125)
                        for i in range(gl):
                            lt = g0 + i
                            nc.tensor.matmul(ctxp[:],
                                             Vone[:, lt, h * 65:h * 65 + 65],
                                             et[:, i * QB:(i + 1) * QB],
                                             start=(lt == 0), stop=(lt == L - 1))
                    # free the PSUM accumulator immediately, normalize off-path
                    stg = small.tile([128, QB], F32, tag="stg")
                    nc.vector.tensor_copy(stg[0:65, :], ctxp[0:65, :])
                    dn = small.tile([1, QB], F32, tag="dn")
                    nc.vector.tensor_copy(dn[:], stg[64:65, :])
                    rec = small.tile([1, QB], F32, tag="rec")
                    nc.vector.reciprocal_approx_fast(rec[:], dn[:])
                    rb = small.tile([64, QB], F32, tag="rb")
                    nc.gpsimd.partition_broadcast(rb[:], rec[:])
                    if h == 1:
                        st2 = small.tile([64, QB], F32R, tag="st2")
                        nc.vector.tensor_mul(st2[:], stg[0:64, :], rb[:])
                        nc.sync.dma_start(ctxT01[64:128, qs], st2[:])
                    else:
                        dst = ctxT01 if h == 0 else ctxT2
                        nc.vector.tensor_mul(dst[0:64, qs], stg[0:64, :], rb[:])
                for oc in range(nct):
                    emit_outproj(qb, oc)

    nc.compile()
    return nc


_NC_CACHE = {}
LAST_EXEC_NS = None
LAST_RES = None


def _get_nc():
    if "full" not in _NC_CACHE:
        _NC_CACHE["full"] = build_kernel()
    return _NC_CACHE["full"]


def _install_ntff_shim():
    """Make run_bass_kernel_spmd(trace=True) work under axon in this image."""
    import antenv
    if "antenv.axon_hooks" in sys.modules:
        return
    mod = types.ModuleType("antenv.axon_hooks")
    mod._hook = None
    mod.set_axon_ntff_profile_hook = lambda h: setattr(mod, "_hook", h)
    mod.get_axon_ntff_profile_hook = lambda: mod._hook
    sys.modules["antenv.axon_hooks"] = mod
    antenv.axon_hooks = mod
    try:
        from trn_agent_boot.trn_boot import _ntff_profile_via_ctypes
        mod.set_axon_ntff_profile_hook(
            _ntff_profile_via_ctypes("/opt/axon/libaxon_pjrt.so"))
    except Exception:
        pass


def make_in_maps(x, wq, wk, wv, wo):
    x = np.asarray(x, dtype=np.float32)
    wq = np.asarray(wq, dtype=np.float32)
    wk = np.asarray(wk, dtype=np.float32)
    wv = np.asarray(wv, dtype=np.float32)
    wo = np.asarray(wo, dtype=np.float32)
    in_maps = []
    for c in range(NCORES):
        b, g = c // (NCORES // B), c % (NCORES // B)
        rs, re = g * DH, (g + 1) * DH
        wvT = np.zeros((C, 256), dtype=np.float32)
        wvT[:, :DH] = wv[rs:re].T
        woT = np.zeros((256, C), dtype=np.float32)
        woT[:DH] = wo[:, rs:re].T
        in_maps.append({
            "xT": np.ascontiguousarray(x[b].T),
            "wqT": np.ascontiguousarray(wq[rs:re].T),
            "wkT": np.ascontiguousarray(wk[rs:re].T),
            "wvT": wvT,
            "woT": woT,
        })
    return in_maps


def kernel(x, wq, wk, wv, wo):
    global LAST_EXEC_NS, LAST_RES
    in_maps = make_in_maps(x, wq, wk, wv, wo)
    nc = _get_nc()
    trace = bool(int(os.environ.get("KERNEL_TRACE", "0")))
    if trace:
        _install_ntff_shim()
    res = run_bass_kernel_spmd(nc, in_maps, core_ids=list(range(NCORES)),
                               trace=trace)
    LAST_EXEC_NS = res.exec_time_ns
    LAST_RES = res
    outT = [res.results[c]["outT"] for c in range(NCORES)]
    halves = []
    for b in range(B):
        acc = outT[4 * b].astype(np.float64)
        for c in range(4 * b + 1, 4 * b + 4):
            acc = acc + outT[c]
        halves.append(acc.T)
    return np.stack(halves).astype(np.float32)


# revision 24
# speedup vs baseline: 1.3766x; 1.1611x over previous
"""Trainium2 Bass kernel: causal multi-head self-attention.

Problem: B=2, T=4096, C=768, H=12, D=64, causal softmax(QK^T/sqrt(D))V + out proj.

Sharding (8 cores): core c handles batch b=c//4 and 3 heads g=c%4 (rows
192*g:192*(g+1) of wq/wk/wv, same columns of wo). Each core computes its
heads' full attention and a partial out-projection (T, C) for its batch;
the host sums the 4 partials per batch (output is sum-sharded across the
tensor-parallel group) and transposes back to (B, T, C).

Device-side dataflow (everything stays transposed; zero on-chip transposes):
  - inputs are fed pre-transposed: xT (C, T), wqT/wkT/wvT (C, 192), woT (256, C)
  - Q^T, K^T (d on partitions) and V (t on partitions) via matmuls over xT
  - scores computed transposed: S^T[l, q] = sum_d K^T[d,l] Q^T[d,q] (PSUM)
  - every matmul keeps contraction K=128 (zero-padded Q / woT rows): the PE
    HAM clock monitor ignores partial-contraction matmuls, so K=64 work runs
    at 1.2 GHz instead of 2.4 GHz — padding with zeros is 2x faster
  - causal: only l-tiles up to the diagonal; diagonal exp tiles multiplied
    by a 0/1 mask on GpSimd (idle engine) after exp
  - exp on ScalarE with scale=1/sqrt(D) folded in; no max-subtraction
    (scores ~ N(0,1), fp32 exp is exact-safe and matches softmax math)
  - ctx^T[dv, q] = sum_l V[l, dv] E[l, q] accumulated in PSUM; V carries an
    extra ones column so row 64 of the PSUM accumulator is the softmax
    denominator; normalize with reciprocal_approx_fast
  - out^T[oc, t] = sum_dh woT[dh, oc] ctx^T[dh, t]

All matmul operands are float32r (full-rate fp32, ~tf32 operand rounding).
"""

import os
import sys
import types

import numpy as np

if "/opt/trn_rl_repo" not in sys.path:
    sys.path.insert(0, "/opt/trn_rl_repo")

import concourse.bass as bass  # noqa: E402
import concourse.mybir as mybir  # noqa: E402
from concourse import bacc, tile  # noqa: E402
from concourse.bass_utils import run_bass_kernel_spmd  # noqa: E402

F32 = mybir.dt.float32
F32R = mybir.dt.float32r
BF16 = mybir.dt.bfloat16
EXP = mybir.ActivationFunctionType.Exp

B, T, C, H, D = 2, 4096, 768, 12, 64
HPD = 3          # heads per device
DH = HPD * D     # 192 local head channels
NCORES = 8
QB = 512         # query block (matmul free dim / PSUM bank)
LT = 128         # key(l)-tile size
GRP = 3          # l-tiles per exp group (3 PSUM banks)


def build_kernel(t=T, trace_sim=False):
    n_lt = t // LT
    n_qb = t // QB
    n_ch = t // QB
    nct = C // 128            # 6

    nc = bacc.Bacc("TRN2", target_bir_lowering=False, debug=False,
                   num_devices=NCORES)
    xT_d = nc.dram_tensor("xT", [C, t], F32R, kind="ExternalInput")
    wqT_d = nc.dram_tensor("wqT", [C, DH], F32R, kind="ExternalInput")
    wkT_d = nc.dram_tensor("wkT", [C, DH], F32R, kind="ExternalInput")
    wvT_d = nc.dram_tensor("wvT", [C, 256], F32R, kind="ExternalInput")  # padded
    woT_d = nc.dram_tensor("woT", [256, C], F32R, kind="ExternalInput")  # padded
    outT_d = nc.dram_tensor("outT", [C, t], F32, kind="ExternalOutput")

    with tile.TileContext(nc, trace_sim=trace_sim) as tc:
        with (
            tc.tile_pool(name="const", bufs=1) as const,
            tc.tile_pool(name="xs", bufs=2) as xs,
            tc.tile_pool(name="epool", bufs=3) as epool,
            tc.tile_pool(name="small", bufs=2) as small,
            tc.tile_pool(name="sp", bufs=2, space="PSUM") as sp,
            tc.tile_pool(name="ps", bufs=2, space="PSUM") as ps,
        ):
            # ---- weights / constants -------------------------------------
            wqT_s = const.tile([128, nct, DH], F32R)
            wkT_s = const.tile([128, nct, DH], F32R)
            wvT_s = const.tile([128, nct, 256], F32R)
            nc.sync.dma_start(wqT_s[:], wqT_d.ap().rearrange("(ct p) d -> p ct d", p=128))
            nc.sync.dma_start(wkT_s[:], wkT_d.ap().rearrange("(ct p) d -> p ct d", p=128))
            nc.sync.dma_start(wvT_s[:], wvT_d.ap().rearrange("(ct p) d -> p ct d", p=128))
            woT_a = const.tile([128, C], F32R)
            woT_b = const.tile([128, C], F32R)   # rows 64:128 are host zeros
            nc.sync.dma_start(woT_a[:], woT_d.ap()[0:128, :])
            nc.sync.dma_start(woT_b[:], woT_d.ap()[128:256, :])

            # additive diagonal causal masks: mask[k][p, f] = 0 if p + 128k <= f
            masks = []
            for k in range(QB // LT):
                m = const.tile([128, QB], F32, tag=f"mask{k}")
                nc.gpsimd.memset(m[:], 0.0)
                nc.gpsimd.affine_select(
                    out=m[:], in_=m[:],
                    compare_op=mybir.AluOpType.is_ge,
                    fill=-1.0e30,
                    base=-128 * k,
                    channel_multiplier=-1,
                    pattern=[[1, QB]],
                )
                mb = const.tile([128, QB], BF16, tag=f"maskb{k}", name=f"maskb{k}")
                nc.vector.tensor_copy(mb[:], m[:])
                masks.append(mb)

            ident = const.tile([128, 128], F32, tag="identf")
            nc.gpsimd.memset(ident[:], 1.0)
            nc.gpsimd.affine_select(
                out=ident[:], in_=ident[:],
                compare_op=mybir.AluOpType.is_equal,
                fill=0.0, base=0, channel_multiplier=1,
                pattern=[[-1, 128]],
            )
            identb = const.tile([128, 128], BF16, tag="identb")
            nc.vector.tensor_copy(identb[:], ident[:])

            ones1 = const.tile([128, 1], F32)
            nc.vector.memset(ones1[:], 1.0)
            zero1 = const.tile([128, 1], F32)
            nc.vector.memset(zero1[:], 0.0)

            # ---- big persistent activations ------------------------------
            # K: heads 0,1 stacked; head 2 alone (top half zeroed)
            KT01 = const.tile([128, t], BF16)
            KT2 = const.tile([128, t], BF16)
            # Q: one tile per head, other-head partition rows zeroed so every
            # scores matmul contracts over the full 128 partitions
            QTz = [const.tile([128, t], BF16, tag=f"qtz{h}", name=f"qtz{h}")
                   for h in range(HPD)]
            # V with ones column per head: [128, n_lt, 3*65]
            Vone = const.tile([128, n_lt, HPD * 65], BF16)
            ctxT01 = const.tile([128, t], F32R)
            ctxT2 = const.tile([128, t], F32R)   # rows 64:128 zeroed

            # zero-fill everything with dead rows (avoids NaN*0 in the PE)
            for buf in (*QTz, KT2, ctxT2):
                nc.vector.tensor_copy(buf[:], zero1[:].to_broadcast((128, t)))
            nc.vector.tensor_copy(
                Vone[:].rearrange("p a b -> p (a b)"),
                ones1[:].to_broadcast((128, n_lt * HPD * 65)))

            # live partition rows per head for Q/ctx
            qrows = [slice(0, 64), slice(64, 128), slice(0, 64)]

            # ---- phase 1: projections ------------------------------------
            xT_r = xT_d.ap().rearrange("(ct p) t -> p ct t", p=128)
            for ch in range(n_ch):
                cs = slice(ch * QB, (ch + 1) * QB)
                xc = xs.tile([128, nct, QB], F32R)
                for ct in range(nct):
                    nc.sync.dma_start(xc[:, ct, :], xT_r[:, ct, cs])

                pqk = sp.tile([128, 3 * QB], F32, tag="sp")
                for ct in range(nct):
                    f, l = (ct == 0), (ct == nct - 1)
                    nc.tensor.matmul(pqk[:, 0:QB], wqT_s[:, ct, 0:128],
                                     xc[:, ct, :], start=f, stop=l)
                    nc.tensor.matmul(pqk[:, QB:2 * QB], wkT_s[:, ct, 0:128],
                                     xc[:, ct, :], start=f, stop=l)
                    nc.tensor.matmul(pqk[0:64, 2 * QB:3 * QB], wqT_s[:, ct, 128:DH],
                                     xc[:, ct, :], start=f, stop=l)
                pk2 = ps.tile([64, QB], F32, tag="ps")
                for ct in range(nct):
                    nc.tensor.matmul(pk2[:], wkT_s[:, ct, 128:DH],
                                     xc[:, ct, :], start=(ct == 0),
                                     stop=(ct == nct - 1))

                # copy projections out of PSUM (all lane-aligned)
                nc.vector.tensor_copy(QTz[0][0:64, cs], pqk[0:64, 0:QB])
                nc.vector.tensor_copy(QTz[1][64:128, cs], pqk[64:128, 0:QB])
                nc.vector.tensor_copy(QTz[2][0:64, cs], pqk[0:64, 2 * QB:3 * QB])
                nc.vector.tensor_copy(KT01[:, cs], pqk[:, QB:2 * QB])
                nc.vector.tensor_copy(KT2[0:64, cs], pk2[:])

                # V natural layout: stationary xT block, stream wvT (padded 256)
                pv = sp.tile([128, 3 * QB], F32, tag="sp")
                for ts in range(QB // 128):
                    tt = ch * (QB // 128) + ts
                    po = pv[:, ts * 256:(ts + 1) * 256]
                    for ct in range(nct):
                        nc.tensor.matmul(po, xc[:, ct, ts * 128:(ts + 1) * 128],
                                         wvT_s[:, ct, :], start=(ct == 0),
                                         stop=(ct == nct - 1))
                    for h in range(HPD):
                        nc.vector.tensor_copy(
                            Vone[:, tt, h * 65:h * 65 + 64],
                            pv[:, ts * 256 + h * 64:ts * 256 + (h + 1) * 64])

            # ---- phase 2: attention + interleaved output projection -------
            def emit_outproj(qb, oc):
                qs = slice(qb * QB, (qb + 1) * QB)
                ocs = slice(oc * 128, (oc + 1) * 128)
                po = ps.tile([128, QB], F32, tag="ps")
                nc.tensor.matmul(po[:], woT_a[:, ocs], ctxT01[:, qs],
                                 start=True, stop=False)
                nc.tensor.matmul(po[:], woT_b[:, ocs], ctxT2[:, qs],
                                 start=False, stop=True)
                ot = small.tile([128, QB], F32, tag="ot")
                nc.vector.tensor_copy(ot[:], po[:])
                nc.sync.dma_start(outT_d.ap()[ocs, qs], ot[:])

            pending = []
            for qb in range(n_qb):
                qs = slice(qb * QB, (qb + 1) * QB)
                L = (qb + 1) * (QB // LT)
                for h in range(HPD):
                    KT_h = KT01 if h < 2 else KT2
                    ctxp = ps.tile([65, QB], F32, tag="ps")
                    for g0 in range(0, L, GRP):
                        gl = min(GRP, L - g0)
                        spt = sp.tile([128, 3 * QB], F32, tag="sp")
                        for i in range(gl):
                            lt = g0 + i
                            sl = spt[:, i * QB:(i + 1) * QB]
                            diag = lt - qb * (QB // LT)
                            if diag >= 0:
                                # causal mask as a PE pre-accumulation:
                                # identity.T @ maskb = the -1e30 step pattern
                                nc.tensor.matmul(sl, identb[:], masks[diag][:],
                                                 start=True, stop=False)
                                nc.tensor.matmul(sl,
                                                 KT_h[:, lt * LT:(lt + 1) * LT],
                                                 QTz[h][:, qs],
                                                 start=False, stop=True)
                            else:
                                nc.tensor.matmul(sl,
                                                 KT_h[:, lt * LT:(lt + 1) * LT],
                                                 QTz[h][:, qs],
                                                 start=True, stop=True)
                        et = epool.tile([128, GRP * QB], BF16)
                        nc.scalar.activation(et[:, :gl * QB], spt[:, :gl * QB],
                                             EXP, scale=0.125)
                        for i in range(gl):
                            lt = g0 + i
                            nc.tensor.matmul(ctxp[:],
                                             Vone[:, lt, h * 65:h * 65 + 65],
                                             et[:, i * QB:(i + 1) * QB],
                                             start=(lt == 0), stop=(lt == L - 1))
                        if pending:
                            emit_outproj(*pending.pop(0))
                    # free the PSUM accumulator immediately, normalize off-path
                    stg = small.tile([128, QB], F32, tag="stg")
                    nc.vector.tensor_copy(stg[0:65, :], ctxp[0:65, :])
                    dn = small.tile([1, QB], F32, tag="dn")
                    nc.vector.tensor_copy(dn[:], stg[64:65, :])
                    rec = small.tile([1, QB], F32, tag="rec")
                    nc.vector.reciprocal_approx_fast(rec[:], dn[:])
                    rb = small.tile([64, QB], F32, tag="rb")
                    nc.gpsimd.partition_broadcast(rb[:], rec[:])
                    if h == 1:
                        st2 = small.tile([64, QB], F32R, tag="st2")
                        nc.vector.tensor_mul(st2[:], stg[0:64, :], rb[:])
                        nc.sync.dma_start(ctxT01[64:128, qs], st2[:])
                    else:
                        dst = ctxT01 if h == 0 else ctxT2
                        nc.vector.tensor_mul(dst[0:64, qs], stg[0:64, :], rb[:])
                pending.extend((qb, oc) for oc in range(nct))
            for item in pending:
                emit_outproj(*item)

    nc.compile()
    return nc


_NC_CACHE = {}
LAST_EXEC_NS = None
LAST_RES = None


def _get_nc():
    if "full" not in _NC_CACHE:
        _NC_CACHE["full"] = build_kernel()
    return _NC_CACHE["full"]


def _install_ntff_shim():
    """Make run_bass_kernel_spmd(trace=True) work under axon in this image."""
    import antenv
    if "antenv.axon_hooks" in sys.modules:
        return
    mod = types.ModuleType("antenv.axon_hooks")
    mod._hook = None
    mod.set_axon_ntff_profile_hook = lambda h: setattr(mod, "_hook", h)
    mod.get_axon_ntff_profile_hook = lambda: mod._hook
    sys.modules["antenv.axon_hooks"] = mod
    antenv.axon_hooks = mod
    try:
        from trn_agent_boot.trn_boot import _ntff_profile_via_ctypes
        mod.set_axon_ntff_profile_hook(
            _ntff_profile_via_ctypes("/opt/axon/libaxon_pjrt.so"))
    except Exception:
        pass


def make_in_maps(x, wq, wk, wv, wo):
    x = np.asarray(x, dtype=np.float32)
    wq = np.asarray(wq, dtype=np.float32)
    wk = np.asarray(wk, dtype=np.float32)
    wv = np.asarray(wv, dtype=np.float32)
    wo = np.asarray(wo, dtype=np.float32)
    in_maps = []
    for c in range(NCORES):
        b, g = c // (NCORES // B), c % (NCORES // B)
        rs, re = g * DH, (g + 1) * DH
        wvT = np.zeros((C, 256), dtype=np.float32)
        wvT[:, :DH] = wv[rs:re].T
        woT = np.zeros((256, C), dtype=np.float32)
        woT[:DH] = wo[:, rs:re].T
        in_maps.append({
            "xT": np.ascontiguousarray(x[b].T),
            "wqT": np.ascontiguousarray(wq[rs:re].T),
            "wkT": np.ascontiguousarray(wk[rs:re].T),
            "wvT": wvT,
            "woT": woT,
        })
    return in_maps


def kernel(x, wq, wk, wv, wo):
    global LAST_EXEC_NS, LAST_RES
    in_maps = make_in_maps(x, wq, wk, wv, wo)
    nc = _get_nc()
    trace = bool(int(os.environ.get("KERNEL_TRACE", "0")))
    if trace:
        _install_ntff_shim()
    res = run_bass_kernel_spmd(nc, in_maps, core_ids=list(range(NCORES)),
                               trace=trace)
    LAST_EXEC_NS = res.exec_time_ns
    LAST_RES = res
    outT = [res.results[c]["outT"] for c in range(NCORES)]
    halves = []
    for b in range(B):
        acc = outT[4 * b].astype(np.float64)
        for c in range(4 * b + 1, 4 * b + 4):
            acc = acc + outT[c]
        halves.append(acc.T)
    return np.stack(halves).astype(np.float32)


# revision 25
# speedup vs baseline: 1.4239x; 1.0344x over previous
"""Trainium2 Bass kernel: causal multi-head self-attention.

Problem: B=2, T=4096, C=768, H=12, D=64, causal softmax(QK^T/sqrt(D))V + out proj.

Sharding (8 cores): core c handles batch b=c//4 and 3 heads g=c%4 (rows
192*g:192*(g+1) of wq/wk/wv, same columns of wo). Each core computes its
heads' full attention and a partial out-projection (T, C) for its batch;
the host sums the 4 partials per batch (output is sum-sharded across the
tensor-parallel group) and transposes back to (B, T, C).

Device-side dataflow (everything stays transposed; zero on-chip transposes):
  - inputs are fed pre-transposed: xT (C, T), wqT/wkT/wvT (C, 192), woT (256, C)
  - Q^T, K^T (d on partitions) and V (t on partitions) via matmuls over xT
  - scores computed transposed: S^T[l, q] = sum_d K^T[d,l] Q^T[d,q] (PSUM)
  - every matmul keeps contraction K=128 (zero-padded Q / woT rows): the PE
    HAM clock monitor ignores partial-contraction matmuls, so K=64 work runs
    at 1.2 GHz instead of 2.4 GHz — padding with zeros is 2x faster
  - causal: only l-tiles up to the diagonal; diagonal exp tiles multiplied
    by a 0/1 mask on GpSimd (idle engine) after exp
  - exp on ScalarE with scale=1/sqrt(D) folded in; no max-subtraction
    (scores ~ N(0,1), fp32 exp is exact-safe and matches softmax math)
  - ctx^T[dv, q] = sum_l V[l, dv] E[l, q] accumulated in PSUM; V carries an
    extra ones column so row 64 of the PSUM accumulator is the softmax
    denominator; normalize with reciprocal_approx_fast
  - out^T[oc, t] = sum_dh woT[dh, oc] ctx^T[dh, t]

All matmul operands are float32r (full-rate fp32, ~tf32 operand rounding).
"""

import os
import sys
import types

import numpy as np

if "/opt/trn_rl_repo" not in sys.path:
    sys.path.insert(0, "/opt/trn_rl_repo")

import concourse.bass as bass  # noqa: E402
import concourse.mybir as mybir  # noqa: E402
from concourse import bacc, tile  # noqa: E402
from concourse.bass_utils import run_bass_kernel_spmd  # noqa: E402

F32 = mybir.dt.float32
F32R = mybir.dt.float32r
BF16 = mybir.dt.bfloat16
EXP = mybir.ActivationFunctionType.Exp

B, T, C, H, D = 2, 4096, 768, 12, 64
HPD = 3          # heads per device
DH = HPD * D     # 192 local head channels
NCORES = 8
QB = 512         # query block (matmul free dim / PSUM bank)
LT = 128         # key(l)-tile size
GRP = 3          # l-tiles per exp group (3 PSUM banks)


def build_kernel(t=T, trace_sim=False):
    n_lt = t // LT
    n_qb = t // QB
    n_ch = t // QB
    nct = C // 128            # 6

    nc = bacc.Bacc("TRN2", target_bir_lowering=False, debug=False,
                   num_devices=NCORES)
    xT_d = nc.dram_tensor("xT", [C, t], F32R, kind="ExternalInput")
    wqT_d = nc.dram_tensor("wqT", [C, DH], F32R, kind="ExternalInput")
    wkT_d = nc.dram_tensor("wkT", [C, DH], F32R, kind="ExternalInput")
    wvT_d = nc.dram_tensor("wvT", [C, 256], F32R, kind="ExternalInput")  # padded
    woT_d = nc.dram_tensor("woT", [256, C], F32R, kind="ExternalInput")  # padded
    outT_d = nc.dram_tensor("outT", [C, t], F32, kind="ExternalOutput")

    with tile.TileContext(nc, trace_sim=trace_sim) as tc:
        with (
            tc.tile_pool(name="const", bufs=1) as const,
            tc.tile_pool(name="xs", bufs=3) as xs,
            tc.tile_pool(name="epool", bufs=3) as epool,
            tc.tile_pool(name="small", bufs=3) as small,
            tc.tile_pool(name="sp", bufs=2, space="PSUM") as sp,
            tc.tile_pool(name="ps", bufs=2, space="PSUM") as ps,
        ):
            # ---- weights / constants -------------------------------------
            wqT_s = const.tile([128, nct, DH], F32R)
            wkT_s = const.tile([128, nct, DH], F32R)
            wvT_s = const.tile([128, nct, 256], F32R)
            nc.sync.dma_start(wqT_s[:], wqT_d.ap().rearrange("(ct p) d -> p ct d", p=128))
            nc.sync.dma_start(wkT_s[:], wkT_d.ap().rearrange("(ct p) d -> p ct d", p=128))
            nc.sync.dma_start(wvT_s[:], wvT_d.ap().rearrange("(ct p) d -> p ct d", p=128))
            woT_a = const.tile([128, C], F32R)
            woT_b = const.tile([128, C], F32R)   # rows 64:128 are host zeros
            nc.sync.dma_start(woT_a[:], woT_d.ap()[0:128, :])
            nc.sync.dma_start(woT_b[:], woT_d.ap()[128:256, :])

            # additive diagonal causal masks: mask[k][p, f] = 0 if p + 128k <= f
            masks = []
            for k in range(QB // LT):
                m = const.tile([128, QB], F32, tag=f"mask{k}")
                nc.gpsimd.memset(m[:], 0.0)
                nc.gpsimd.affine_select(
                    out=m[:], in_=m[:],
                    compare_op=mybir.AluOpType.is_ge,
                    fill=-1.0e30,
                    base=-128 * k,
                    channel_multiplier=-1,
                    pattern=[[1, QB]],
                )
                mb = const.tile([128, QB], BF16, tag=f"maskb{k}", name=f"maskb{k}")
                nc.vector.tensor_copy(mb[:], m[:])
                masks.append(mb)

            ident = const.tile([128, 128], F32, tag="identf")
            nc.gpsimd.memset(ident[:], 1.0)
            nc.gpsimd.affine_select(
                out=ident[:], in_=ident[:],
                compare_op=mybir.AluOpType.is_equal,
                fill=0.0, base=0, channel_multiplier=1,
                pattern=[[-1, 128]],
            )
            identb = const.tile([128, 128], BF16, tag="identb")
            nc.vector.tensor_copy(identb[:], ident[:])

            ones1 = const.tile([128, 1], F32)
            nc.vector.memset(ones1[:], 1.0)
            zero1 = const.tile([128, 1], F32)
            nc.vector.memset(zero1[:], 0.0)

            # ---- big persistent activations ------------------------------
            # K: heads 0,1 stacked; head 2 alone (top half zeroed)
            KT01 = const.tile([128, t], BF16)
            KT2 = const.tile([128, t], BF16)
            # Q: one tile per head, other-head partition rows zeroed so every
            # scores matmul contracts over the full 128 partitions
            QTz = [const.tile([128, t], BF16, tag=f"qtz{h}", name=f"qtz{h}")
                   for h in range(HPD)]
            # V with ones column per head: [128, n_lt, 3*65]
            Vone = const.tile([128, n_lt, HPD * 65], BF16)
            ctxT01 = const.tile([128, t], F32R)
            ctxT2 = const.tile([128, t], F32R)   # rows 64:128 zeroed

            # zero-fill everything with dead rows (avoids NaN*0 in the PE)
            for buf in (*QTz, KT2, ctxT2):
                nc.vector.tensor_copy(buf[:], zero1[:].to_broadcast((128, t)))
            nc.vector.tensor_copy(
                Vone[:].rearrange("p a b -> p (a b)"),
                ones1[:].to_broadcast((128, n_lt * HPD * 65)))

            # live partition rows per head for Q/ctx
            qrows = [slice(0, 64), slice(64, 128), slice(0, 64)]

            # ---- phase 1: projections ------------------------------------
            xT_r = xT_d.ap().rearrange("(ct p) t -> p ct t", p=128)
            for ch in range(n_ch):
                cs = slice(ch * QB, (ch + 1) * QB)
                xc = xs.tile([128, nct, QB], F32R)
                for ct in range(nct):
                    nc.sync.dma_start(xc[:, ct, :], xT_r[:, ct, cs])

                pqk = sp.tile([128, 3 * QB], F32, tag="sp")
                for ct in range(nct):
                    f, l = (ct == 0), (ct == nct - 1)
                    nc.tensor.matmul(pqk[:, 0:QB], wqT_s[:, ct, 0:128],
                                     xc[:, ct, :], start=f, stop=l)
                    nc.tensor.matmul(pqk[:, QB:2 * QB], wkT_s[:, ct, 0:128],
                                     xc[:, ct, :], start=f, stop=l)
                    nc.tensor.matmul(pqk[0:64, 2 * QB:3 * QB], wqT_s[:, ct, 128:DH],
                                     xc[:, ct, :], start=f, stop=l)
                pk2 = ps.tile([64, QB], F32, tag="ps")
                for ct in range(nct):
                    nc.tensor.matmul(pk2[:], wkT_s[:, ct, 128:DH],
                                     xc[:, ct, :], start=(ct == 0),
                                     stop=(ct == nct - 1))

                # copy projections out of PSUM (all lane-aligned)
                nc.vector.tensor_copy(QTz[0][0:64, cs], pqk[0:64, 0:QB])
                nc.vector.tensor_copy(QTz[1][64:128, cs], pqk[64:128, 0:QB])
                nc.vector.tensor_copy(QTz[2][0:64, cs], pqk[0:64, 2 * QB:3 * QB])
                nc.vector.tensor_copy(KT01[:, cs], pqk[:, QB:2 * QB])
                nc.vector.tensor_copy(KT2[0:64, cs], pk2[:])

                # V natural layout: stationary xT block, stream wvT (padded 256)
                pv = sp.tile([128, 3 * QB], F32, tag="sp")
                for ts in range(QB // 128):
                    tt = ch * (QB // 128) + ts
                    po = pv[:, ts * 256:(ts + 1) * 256]
                    for ct in range(nct):
                        nc.tensor.matmul(po, xc[:, ct, ts * 128:(ts + 1) * 128],
                                         wvT_s[:, ct, :], start=(ct == 0),
                                         stop=(ct == nct - 1))
                    for h in range(HPD):
                        nc.vector.tensor_copy(
                            Vone[:, tt, h * 65:h * 65 + 64],
                            pv[:, ts * 256 + h * 64:ts * 256 + (h + 1) * 64])

            # ---- phase 2: attention + interleaved output projection -------
            def emit_outproj(qb, oc):
                qs = slice(qb * QB, (qb + 1) * QB)
                ocs = slice(oc * 128, (oc + 1) * 128)
                po = ps.tile([128, QB], F32, tag="ps")
                nc.tensor.matmul(po[:], woT_a[:, ocs], ctxT01[:, qs],
                                 start=True, stop=False)
                nc.tensor.matmul(po[:], woT_b[:, ocs], ctxT2[:, qs],
                                 start=False, stop=True)
                ot = small.tile([128, QB], F32, tag="ot")
                nc.vector.tensor_copy(ot[:], po[:])
                nc.sync.dma_start(outT_d.ap()[ocs, qs], ot[:])

            pending = []
            for qb in range(n_qb):
                qs = slice(qb * QB, (qb + 1) * QB)
                L = (qb + 1) * (QB // LT)
                for h in range(HPD):
                    KT_h = KT01 if h < 2 else KT2
                    ctxp = ps.tile([65, QB], F32, tag="ps")
                    for g0 in range(0, L, GRP):
                        gl = min(GRP, L - g0)
                        spt = sp.tile([128, 3 * QB], F32, tag="sp")
                        for i in range(gl):
                            lt = g0 + i
                            sl = spt[:, i * QB:(i + 1) * QB]
                            diag = lt - qb * (QB // LT)
                            if diag >= 0:
                                # causal mask as a PE pre-accumulation:
                                # identity.T @ maskb = the -1e30 step pattern
                                nc.tensor.matmul(sl, identb[:], masks[diag][:],
                                                 start=True, stop=False)
                                nc.tensor.matmul(sl,
                                                 KT_h[:, lt * LT:(lt + 1) * LT],
                                                 QTz[h][:, qs],
                                                 start=False, stop=True)
                            else:
                                nc.tensor.matmul(sl,
                                                 KT_h[:, lt * LT:(lt + 1) * LT],
                                                 QTz[h][:, qs],
                                                 start=True, stop=True)
                        et = epool.tile([128, GRP * QB], BF16)
                        nc.scalar.activation(et[:, :gl * QB], spt[:, :gl * QB],
                                             EXP, scale=0.125)
                        for i in range(gl):
                            lt = g0 + i
                            nc.tensor.matmul(ctxp[:],
                                             Vone[:, lt, h * 65:h * 65 + 65],
                                             et[:, i * QB:(i + 1) * QB],
                                             start=(lt == 0), stop=(lt == L - 1))
                        if pending:
                            emit_outproj(*pending.pop(0))
                    # free the PSUM accumulator immediately, normalize off-path
                    stg = small.tile([128, QB], F32, tag="stg")
                    nc.vector.tensor_copy(stg[0:65, :], ctxp[0:65, :])
                    dn = small.tile([1, QB], F32, tag="dn")
                    nc.vector.tensor_copy(dn[:], stg[64:65, :])
                    rec = small.tile([1, QB], F32, tag="rec")
                    nc.vector.reciprocal_approx_fast(rec[:], dn[:])
                    rb = small.tile([64, QB], F32, tag="rb")
                    nc.gpsimd.partition_broadcast(rb[:], rec[:])
                    if h == 1:
                        st2 = small.tile([64, QB], F32R, tag="st2")
                        nc.vector.tensor_mul(st2[:], stg[0:64, :], rb[:])
                        nc.sync.dma_start(ctxT01[64:128, qs], st2[:])
                    else:
                        dst = ctxT01 if h == 0 else ctxT2
                        nc.vector.tensor_mul(dst[0:64, qs], stg[0:64, :], rb[:])
                pending.extend((qb, oc) for oc in range(nct))
            for item in pending:
                emit_outproj(*item)

    nc.compile()
    return nc


_NC_CACHE = {}
LAST_EXEC_NS = None
LAST_RES = None


def _get_nc():
    if "full" not in _NC_CACHE:
        _NC_CACHE["full"] = build_kernel()
    return _NC_CACHE["full"]


def _install_ntff_shim():
    """Make run_bass_kernel_spmd(trace=True) work under axon in this image."""
    import antenv
    if "antenv.axon_hooks" in sys.modules:
        return
    mod = types.ModuleType("antenv.axon_hooks")
    mod._hook = None
    mod.set_axon_ntff_profile_hook = lambda h: setattr(mod, "_hook", h)
    mod.get_axon_ntff_profile_hook = lambda: mod._hook
    sys.modules["antenv.axon_hooks"] = mod
    antenv.axon_hooks = mod
    try:
        from trn_agent_boot.trn_boot import _ntff_profile_via_ctypes
        mod.set_axon_ntff_profile_hook(
            _ntff_profile_via_ctypes("/opt/axon/libaxon_pjrt.so"))
    except Exception:
        pass


def make_in_maps(x, wq, wk, wv, wo):
    x = np.asarray(x, dtype=np.float32)
    wq = np.asarray(wq, dtype=np.float32)
    wk = np.asarray(wk, dtype=np.float32)
    wv = np.asarray(wv, dtype=np.float32)
    wo = np.asarray(wo, dtype=np.float32)
    in_maps = []
    for c in range(NCORES):
        b, g = c // (NCORES // B), c % (NCORES // B)
        rs, re = g * DH, (g + 1) * DH
        wvT = np.zeros((C, 256), dtype=np.float32)
        wvT[:, :DH] = wv[rs:re].T
        woT = np.zeros((256, C), dtype=np.float32)
        woT[:DH] = wo[:, rs:re].T
        in_maps.append({
            "xT": np.ascontiguousarray(x[b].T),
            "wqT": np.ascontiguousarray(wq[rs:re].T),
            "wkT": np.ascontiguousarray(wk[rs:re].T),
            "wvT": wvT,
            "woT": woT,
        })
    return in_maps


def kernel(x, wq, wk, wv, wo):
    global LAST_EXEC_NS, LAST_RES
    in_maps = make_in_maps(x, wq, wk, wv, wo)
    nc = _get_nc()
    trace = bool(int(os.environ.get("KERNEL_TRACE", "0")))
    if trace:
        _install_ntff_shim()
    res = run_bass_kernel_spmd(nc, in_maps, core_ids=list(range(NCORES)),
                               trace=trace)
    LAST_EXEC_NS = res.exec_time_ns
    LAST_RES = res
    outT = [res.results[c]["outT"] for c in range(NCORES)]
    halves = []
    for b in range(B):
        acc = outT[4 * b].astype(np.float64)
        for c in range(4 * b + 1, 4 * b + 4):
            acc = acc + outT[c]
        halves.append(acc.T)
    return np.stack(halves).astype(np.float32)
